# revision 18
# baseline (speedup 1.0000x reference)
"""Trainium2 Bass kernel for nn_CrossAttention (elementwise 'attention' transformer block).

Computation (per reference):
  ln(t) = LayerNorm(t, g, b) for t in {x, y, iy2x, ix2y}
  qkv_t = ln(t) @ Wqkv  -> q,k,v  [B, H, hd]   (t in {x, y})
  block(q,k,v,res): attn = softmax(q*k*scale, axis=-1)
                    f = (attn*v) @ Wpb + bpb; f = f.reshape(B,C) + res; f = LN(f)
                    out = f @ Wproj + bproj + f
  outputs: block(qx,kx,vx,ln x), block(qy,ky,vy,ln y),
           block(qy,kx,vx,ln iy2x), block(qx,ky,vy,ln ix2y)

Host-side algebraic folds (weights are tiny, done in numpy):
  W1   = diag(g) @ Wqkv                  (qkv = n_t @ W1 + c1, c1 = b @ Wqkv)
  Wpb~ = block_diag(Wpb x 8)             (per-head matmul as one 768x768)
  W2   = diag(g) @ (Wproj + I)           (out = n_u @ W2 + c2)
  c2   = b @ Wproj + bproj + b
  softmax denominator folded post-Wpb:   u = ((e*v) @ Wpb~) * (1/s)_head + res
where n_t is the pure (g=1,b=0) layernorm normalization.

Device layout: row-major [tokens(partitions) x C(free)] activations; PE
transposes (via identity matmul) produce the [C x tokens] operands each
matmul round needs. LayerNorm stats via bn_stats/bn_aggr; rsqrt via
DVE-only Newton iterations (no ACT table-set thrash with softmax's Exp).

Sharding: pure data-parallel over B across 8 NeuronCores; weights replicated.
"""

import os
import sys

import numpy as np

for _p in ("/opt/trn_rl_repo",):
    if os.path.isdir(_p) and _p not in sys.path:
        sys.path.insert(0, _p)

import concourse.bass as bass
import concourse.tile as tile
from concourse import bacc
from concourse import mybir
from concourse.bass_utils import run_bass_kernel_spmd
from concourse.masks import make_identity

F32 = mybir.dt.float32
F32R = mybir.dt.float32r
I32 = mybir.dt.int32
AF = mybir.ActivationFunctionType
OP = mybir.AluOpType
AX = mybir.AxisListType

N_CORES = 8
B_FULL = 16384
C = 768
H = 8
HD = 96
C3 = 3 * C
EPS = 1e-6
SCALE = float(HD) ** -0.5
P = 128            # token tile (partition dim)
KC = C // P        # 6 contraction chunks
NCH = 384          # psum free chunk (1 bank = 512 f32; 384 = 4 heads)

# matmul streaming dtype: float32r streams at bf16 rate on TRN2 for N>=256
MM_DT = F32R
TP_DT = F32        # PE-transpose dtype (plain f32: no f32r producer-rounding rule)


def _mm(ap):
    return ap


def _tp(ap):
    return ap.bitcast(TP_DT) if TP_DT is not F32 else ap


def _bn_stats(nc, pool, t_ap, tag, sub=256):
    """Full-row mean/var of [P, C] via bn_stats subgroups. Returns mv [P, 2]."""
    nsub = C // sub
    stats = pool.tile([P, nsub, 6], F32, tag=f"bn_stats_{tag}")
    view = t_ap.rearrange("p (s d) -> p s d", s=nsub)
    for s in range(nsub):
        nc.vector.bn_stats(out=stats[:, s, :], in_=view[:, s, :])
    mv = pool.tile([P, 2], F32, tag=f"bn_mv_{tag}")
    nc.vector.bn_aggr(out=mv, in_=stats)
    return mv


def _rsqrt_newton(nc, pool, v_ap, nb, iters=2):
    """rstd [P, nb] = 1/sqrt(v_ap + EPS), DVE-only (quake seed + Newton)."""
    v = pool.tile([P, nb], F32, tag="rs_v")
    nc.vector.tensor_scalar_add(out=v, in0=v_ap, scalar1=float(EPS))
    sh = pool.tile([P, nb], I32, tag="rs_i")
    nc.vector.tensor_scalar(
        out=sh, in0=v.bitcast(I32), scalar1=1, scalar2=None,
        op0=OP.logical_shift_right,
    )
    seed = pool.tile([P, nb], I32, tag="rs_s")
    nc.vector.tensor_scalar(
        out=seed, in0=sh, scalar1=-1, scalar2=0x5F3759DF,
        op0=OP.mult, op1=OP.add,
    )
    y = seed.bitcast(F32)
    t1 = pool.tile([P, nb], F32, tag="rs_t1")
    t2 = pool.tile([P, nb], F32, tag="rs_t2")
    for _ in range(iters):
        nc.vector.tensor_mul(out=t1, in0=y, in1=y)          # y^2
        nc.vector.tensor_mul(out=t2, in0=t1, in1=v)         # v y^2
        nc.vector.tensor_scalar(                            # 1.5 - 0.5 v y^2
            out=t2, in0=t2, scalar1=-0.5, scalar2=1.5, op0=OP.mult, op1=OP.add,
        )
        nc.vector.tensor_mul(out=t1, in0=y, in1=t2)
        nc.vector.tensor_copy(out=y, in_=t1)
    out = pool.tile([P, nb], F32, tag="rs_out")
    nc.vector.tensor_copy(out=out, in_=y)
    return out


def _normalize(nc, pool, t_ap, mv, rstd, tag):
    """n = (t - mean) * rstd  -> new [P, C] tile (DVE tensor_scalar)."""
    n = pool.tile([P, C], F32, tag=tag)
    nc.vector.tensor_scalar(
        out=n, in0=t_ap, scalar1=mv[:, 0:1], scalar2=rstd,
        op0=OP.subtract, op1=OP.mult,
    )
    return n


def _transpose_768(nc, psum_pool, sb_pool, src_ap, ident, tag):
    """PE-transpose a [P(tokens), C] tile into [P(C-chunk), KC, P(tokens)].

    Result chunk k holds src[:, 128k:128k+128].T; evictions via ScalarE.
    """
    dst = sb_pool.tile([P, KC, P], MM_DT, tag=tag)
    for g in range(2):  # two groups of 3 chunks -> one psum bank each
        tp = psum_pool.tile([P, 3 * P], F32, tag="tp_psum")
        for j in range(3):
            k = 3 * g + j
            nc.tensor.transpose(
                _tp(tp[:, j * P:(j + 1) * P]),
                _tp(src_ap[:, k * P:(k + 1) * P]),
                _tp(ident),
            )
        nc.scalar.copy(out=dst[:, 3 * g:3 * g + 3, :], in_=tp)
    return dst


def build_nc(bshard, flags):
    nc = bacc.Bacc()
    ntiles = bshard // P
    assert bshard % P == 0

    has_c1 = flags["has_c1"]
    has_gb = flags["has_gb"]
    has_bpb = flags["has_bpb"]
    has_c2 = flags["has_c2"]

    xd = nc.dram_tensor("x", [bshard, C], F32, kind="ExternalInput")
    yd = nc.dram_tensor("y", [bshard, C], F32, kind="ExternalInput")
    r1d = nc.dram_tensor("r1", [bshard, C], F32, kind="ExternalInput")
    r2d = nc.dram_tensor("r2", [bshard, C], F32, kind="ExternalInput")
    w1d = nc.dram_tensor("w1", [C, C3], MM_DT, kind="ExternalInput")
    wpbd = nc.dram_tensor("wpb", [HD, HD], MM_DT, kind="ExternalInput")
    w2d = nc.dram_tensor("w2", [C, C], MM_DT, kind="ExternalInput")
    if has_c1:
        c1d = nc.dram_tensor("c1", [C3], F32, kind="ExternalInput")
    if has_gb:
        gd = nc.dram_tensor("lng", [C], F32, kind="ExternalInput")
        bd = nc.dram_tensor("lnb", [C], F32, kind="ExternalInput")
    if has_bpb:
        bpbd = nc.dram_tensor("bpbr", [C], F32, kind="ExternalInput")
    if has_c2:
        c2d = nc.dram_tensor("c2", [C], F32, kind="ExternalInput")

    outs_d = [
        nc.dram_tensor(n, [bshard, C], F32, kind="ExternalOutput")
        for n in ("out_x", "out_y", "out_y2x", "out_x2y")
    ]

    lean = has_c1 or has_gb or has_bpb or has_c2
    with tile.TileContext(nc) as tc:
        with (
            tc.tile_pool(name="wts", bufs=1) as wts,
            tc.tile_pool(name="io", bufs=2) as io,
            tc.tile_pool(name="nt", bufs=1 if lean else 2) as ntp,
            tc.tile_pool(name="qkv", bufs=1 if lean else 2) as qkvp,
            tc.tile_pool(name="blk", bufs=2 if lean else 3) as blk,
            tc.tile_pool(name="tiny", bufs=3) as tiny,
            tc.tile_pool(name="outp", bufs=2) as outp,
            tc.tile_pool(name="psum_tp", bufs=2, space="PSUM") as psum_tp,
            tc.tile_pool(name="psum_pb", bufs=1, space="PSUM") as psum_pb,
            tc.tile_pool(name="psum_mm", bufs=2, space="PSUM") as psum_mm,
        ):
            # ---- persistent weights ----
            w1_sb = []
            for k in range(KC):
                w1_chunk = wts.tile([P, C3], MM_DT, tag=f"w1_{k}")
                w1_sb.append(w1_chunk)
            wpb_sb = wts.tile([P, HD], MM_DT)
            w2_sb = wts.tile([P, KC, C], MM_DT)
            for k in range(KC):
                nc.sync.dma_start(out=w1_sb[k], in_=w1d[k * P:(k + 1) * P, :])
            nc.sync.dma_start(out=wpb_sb[0:HD, :], in_=wpbd[:, :])
            nc.sync.dma_start(out=w2_sb, in_=w2d.rearrange("(k p) c -> p k c", p=P))
            ident = wts.tile([P, P], F32)
            make_identity(nc, ident)

            def bcast_row(src, width, tag):
                t = wts.tile([P, width], F32, tag=tag)
                src_b = bass.AP(
                    tensor=src.tensor, offset=src.offset,
                    ap=[[0, P]] + src.ap,
                )
                nc.gpsimd.dma_start(out=t, in_=src_b)
                return t

            c1_sb = bcast_row(c1d[:], C3, "c1b") if has_c1 else None
            g_sb = bcast_row(gd[:], C, "gb") if has_gb else None
            b_sb = bcast_row(bd[:], C, "bb") if has_gb else None
            bpb_sb = bcast_row(bpbd[:], C, "bpbb") if has_bpb else None
            c2_sb = bcast_row(c2d[:], C, "c2b") if has_c2 else None

            for it in range(ntiles):
                rows = slice(it * P, (it + 1) * P)

                # ---- load + layernorm the four inputs ----
                ins = []
                for nm, d in (("x", xd), ("y", yd), ("r1", r1d), ("r2", r2d)):
                    t = io.tile([P, C], F32, tag=f"in_{nm}")
                    nc.sync.dma_start(out=t, in_=d[rows, :])
                    ins.append(t)

                mvs = [_bn_stats(nc, tiny, t, tag=str(j)) for j, t in enumerate(ins)]
                var4 = tiny.tile([P, 4], F32, tag="var4")
                for j in range(4):
                    nc.vector.tensor_copy(out=var4[:, j:j + 1], in_=mvs[j][:, 1:2])
                rstd4 = _rsqrt_newton(nc, tiny, var4, 4)
                for j, (t, mv) in enumerate(zip(ins, mvs)):
                    nc.vector.tensor_scalar(
                        out=t, in0=t, scalar1=mv[:, 0:1], scalar2=rstd4[:, j:j + 1],
                        op0=OP.subtract, op1=OP.mult,
                    )
                n_in = ins
                n_x, n_y, n_r1, n_r2 = n_in

                # residuals (apply g,b only if present)
                if has_gb:
                    res_in = []
                    for j, n in enumerate(n_in):
                        r = blk.tile([P, C], F32, tag=f"res_{j}")
                        nc.vector.tensor_mul(out=r, in0=n, in1=g_sb)
                        nc.vector.tensor_add(out=r, in0=r, in1=b_sb)
                        res_in.append(r)
                else:
                    res_in = n_in
                res_x, res_y, res_r1, res_r2 = res_in

                # ---- qkv for x and y ----
                nxt = _transpose_768(nc, psum_tp, ntp, n_x, ident, "nxT")
                nyt = _transpose_768(nc, psum_tp, ntp, n_y, ident, "nyT")

                qkv = {}
                for nm, nt in (("x", nxt), ("y", nyt)):
                    q = qkvp.tile([P, C3], F32, tag=f"qkv_{nm}")
                    for nchunk in range(C3 // NCH):
                        ncol = slice(nchunk * NCH, (nchunk + 1) * NCH)
                        ps = psum_mm.tile([P, NCH], F32, tag="mm_psum")
                        for k in range(KC):
                            nc.tensor.matmul(
                                ps, _mm(nt[:, k, :]), _mm(w1_sb[k][:, ncol]),
                                start=(k == 0), stop=(k == KC - 1),
                            )
                        if has_c1:
                            nc.vector.tensor_add(out=q[:, ncol], in0=ps, in1=c1_sb[:, ncol])
                        else:
                            nc.scalar.copy(out=q[:, ncol], in_=ps)
                    qkv[nm] = q

                # ---- four attention blocks ----
                specs = [
                    ("x", "x", res_x, outs_d[0]),
                    ("y", "y", res_y, outs_d[1]),
                    ("y", "x", res_r1, outs_d[2]),
                    ("x", "y", res_r2, outs_d[3]),
                ]
                for bi, (qs, kvs, res, od) in enumerate(specs):
                    qa = qkv[qs][:, 0:C]
                    ka = qkv[kvs][:, C:2 * C]
                    va = qkv[kvs][:, 2 * C:3 * C]

                    e = blk.tile([P, C], F32, tag="e")
                    nc.vector.tensor_mul(out=e, in0=qa, in1=ka)
                    nc.scalar.activation(out=e, in_=e, func=AF.Exp, scale=SCALE)

                    s = tiny.tile([P, H], F32, tag="s_sum")
                    nc.vector.reduce_sum(
                        out=s, in_=e.rearrange("p (h d) -> p h d", h=H), axis=AX.X,
                    )
                    rs = tiny.tile([P, H], F32, tag="s_rec")
                    nc.vector.reciprocal(out=rs, in_=s)

                    nc.vector.tensor_mul(out=e, in0=e, in1=va)  # ev, in place

                    # per-head transpose: evT [96, H, 128] (tokens on free dim)
                    evt = blk.tile([P, H, P], MM_DT, tag="evT")
                    for g in range(2):
                        tp = psum_tp.tile([P, 4 * P], F32, tag="tp_psum_h")
                        for j in range(4):
                            h = 4 * g + j
                            nc.tensor.transpose(
                                _tp(tp[0:HD, j * P:(j + 1) * P]),
                                _tp(e[:, h * HD:(h + 1) * HD]),
                                _tp(ident),
                            )
                        nc.scalar.copy(
                            out=evt[0:HD, 4 * g:4 * g + 4, :], in_=tp[0:HD, :]
                        )

                    ps = psum_pb.tile([P, H * P], F32, tag="pb_psum")
                    for h in range(H):
                        nc.tensor.matmul(
                            ps[:, h * P:h * P + HD],
                            _mm(evt[0:HD, h, :]), _mm(wpb_sb[0:HD, :]),
                            start=True, stop=True,
                        )
                    u = blk.tile([P, C], F32, tag="u")
                    rs_b = rs.unsqueeze(2).to_broadcast((P, H, HD))
                    nc.vector.tensor_mul(
                        out=u.rearrange("p (h d) -> p h d", d=HD),
                        in0=ps.rearrange("p (h x) -> p h x", h=H)[:, :, 0:HD],
                        in1=rs_b,
                    )
                    if has_bpb:
                        nc.vector.tensor_add(out=u, in0=u, in1=bpb_sb)
                    nc.vector.tensor_add(out=u, in0=u, in1=res)

                    mv_u = _bn_stats(nc, tiny, u, tag="u")
                    rstd_u = _rsqrt_newton(nc, tiny, mv_u[:, 1:2], 1)
                    nc.vector.tensor_scalar(
                        out=u, in0=u, scalar1=mv_u[:, 0:1], scalar2=rstd_u,
                        op0=OP.subtract, op1=OP.mult,
                    )
                    nut = _transpose_768(nc, psum_tp, blk, u, ident, "nuT")

                    o = outp.tile([P, C], F32, tag="o")
                    for nchunk in range(C // NCH):
                        ncol = slice(nchunk * NCH, (nchunk + 1) * NCH)
                        ps = psum_mm.tile([P, NCH], F32, tag="mm_psum")
                        for k in range(KC):
                            nc.tensor.matmul(
                                ps, _mm(nut[:, k, :]), _mm(w2_sb[:, k, ncol]),
                                start=(k == 0), stop=(k == KC - 1),
                            )
                        if has_c2:
                            nc.vector.tensor_add(out=o[:, ncol], in0=ps, in1=c2_sb[:, ncol])
                        else:
                            nc.scalar.copy(out=o[:, ncol], in_=ps)
                    nc.sync.dma_start(out=od[rows, :], in_=o)

    nc.compile()
    return nc


def _host_prep(x, y, inial_y2x, inial_x2y, Wqkv, Wpb, bpb, Wproj, bproj, ln_g, ln_b):
    g = np.asarray(ln_g, np.float64)
    b = np.asarray(ln_b, np.float64)
    Wqkv64 = np.asarray(Wqkv, np.float64)
    Wproj64 = np.asarray(Wproj, np.float64)

    W1 = (g[:, None] * Wqkv64).astype(np.float32)
    c1 = (b @ Wqkv64).astype(np.float32)
    Wpb_small = np.asarray(Wpb, np.float32)
    bpb_rep = np.tile(np.asarray(bpb, np.float32), H)
    W2 = (g[:, None] * (Wproj64 + np.eye(C))).astype(np.float32)
    c2 = (b @ Wproj64 + np.asarray(bproj, np.float64) + b).astype(np.float32)

    flags = {
        "has_c1": bool(np.any(c1 != 0)),
        "has_gb": bool(np.any(g != 1.0) or np.any(b != 0.0)),
        "has_bpb": bool(np.any(bpb_rep != 0)),
        "has_c2": bool(np.any(c2 != 0)),
    }
    consts = {
        "w1": W1, "wpb": Wpb_small, "w2": W2,
        "c1": c1, "lng": np.asarray(ln_g, np.float32),
        "lnb": np.asarray(ln_b, np.float32),
        "bpbr": bpb_rep, "c2": c2,
    }
    return flags, consts


def _make_in_maps(flags, consts, x, y, r1, r2, n_cores):
    bshard = x.shape[0] // n_cores
    in_maps = []
    for i in range(n_cores):
        sl = slice(i * bshard, (i + 1) * bshard)
        m = {
            "x": np.ascontiguousarray(x[sl], np.float32),
            "y": np.ascontiguousarray(y[sl], np.float32),
            "r1": np.ascontiguousarray(r1[sl], np.float32),
            "r2": np.ascontiguousarray(r2[sl], np.float32),
            "w1": consts["w1"], "wpb": consts["wpb"], "w2": consts["w2"],
        }
        if flags["has_c1"]:
            m["c1"] = consts["c1"]
        if flags["has_gb"]:
            m["lng"], m["lnb"] = consts["lng"], consts["lnb"]
        if flags["has_bpb"]:
            m["bpbr"] = consts["bpbr"]
        if flags["has_c2"]:
            m["c2"] = consts["c2"]
        in_maps.append(m)
    return in_maps, bshard


_KERNEL_CACHE = {}


def kernel(x, y, inial_y2x, inial_x2y, Wqkv, Wpb, bpb, Wproj, bproj, ln_g, ln_b,
           _trace=False):
    x = np.asarray(x, np.float32)
    y = np.asarray(y, np.float32)
    r1 = np.asarray(inial_y2x, np.float32)
    r2 = np.asarray(inial_x2y, np.float32)

    flags, consts = _host_prep(x, y, r1, r2, Wqkv, Wpb, bpb, Wproj, bproj, ln_g, ln_b)
    in_maps, bshard = _make_in_maps(flags, consts, x, y, r1, r2, N_CORES)

    key = (bshard, tuple(sorted(flags.items())))
    if key not in _KERNEL_CACHE:
        _KERNEL_CACHE[key] = build_nc(bshard, flags)
    nc = _KERNEL_CACHE[key]

    res = run_bass_kernel_spmd(nc, in_maps, list(range(N_CORES)), trace=_trace)
    outs = []
    for nm in ("out_x", "out_y", "out_y2x", "out_x2y"):
        outs.append(np.concatenate([res.results[i][nm] for i in range(N_CORES)], axis=0))
    if _trace:
        kernel._last_exec_time_ns = res.exec_time_ns
        kernel._last_results = res
    return tuple(outs)


# revision 23
# speedup vs baseline: 1.0644x; 1.0644x over previous
"""Trainium2 Bass kernel for nn_CrossAttention (elementwise 'attention' transformer block).

Computation (per reference):
  ln(t) = LayerNorm(t, g, b) for t in {x, y, iy2x, ix2y}
  qkv_t = ln(t) @ Wqkv  -> q,k,v  [B, H, hd]   (t in {x, y})
  block(q,k,v,res): attn = softmax(q*k*scale, axis=-1)
                    f = (attn*v) @ Wpb + bpb; f = f.reshape(B,C) + res; f = LN(f)
                    out = f @ Wproj + bproj + f
  outputs: block(qx,kx,vx,ln x), block(qy,ky,vy,ln y),
           block(qy,kx,vx,ln iy2x), block(qx,ky,vy,ln ix2y)

Host-side algebraic folds (weights are tiny, done in numpy):
  W1   = diag(g) @ Wqkv                  (qkv = n_t @ W1 + c1, c1 = b @ Wqkv)
  Wpb~ = block_diag(Wpb x 8)             (per-head matmul as one 768x768)
  W2   = diag(g) @ (Wproj + I)           (out = n_u @ W2 + c2)
  c2   = b @ Wproj + bproj + b
  softmax denominator folded post-Wpb:   u = ((e*v) @ Wpb~) * (1/s)_head + res
where n_t is the pure (g=1,b=0) layernorm normalization.

Device layout: row-major [tokens(partitions) x C(free)] activations; PE
transposes (via identity matmul) produce the [C x tokens] operands each
matmul round needs. LayerNorm stats via bn_stats/bn_aggr; rsqrt via
DVE-only Newton iterations (no ACT table-set thrash with softmax's Exp).

Sharding: pure data-parallel over B across 8 NeuronCores; weights replicated.
"""

import os
import sys

import numpy as np

for _p in ("/opt/trn_rl_repo",):
    if os.path.isdir(_p) and _p not in sys.path:
        sys.path.insert(0, _p)

import concourse.bass as bass
import concourse.tile as tile
from concourse import bacc
from concourse import mybir
from concourse.bass_utils import run_bass_kernel_spmd
from concourse.masks import make_identity

F32 = mybir.dt.float32
F32R = mybir.dt.float32r
I32 = mybir.dt.int32
AF = mybir.ActivationFunctionType
OP = mybir.AluOpType
AX = mybir.AxisListType

N_CORES = 8
B_FULL = 16384
C = 768
H = 8
HD = 96
C3 = 3 * C
EPS = 1e-6
SCALE = float(HD) ** -0.5
P = 128            # token tile (partition dim)
KC = C // P        # 6 contraction chunks
NCH = 384          # psum free chunk (1 bank = 512 f32; 384 = 4 heads)

# matmul streaming dtype: float32r streams at bf16 rate on TRN2 for N>=256
MM_DT = F32R
TP_DT = F32        # PE-transpose dtype (plain f32: no f32r producer-rounding rule)


def _mm(ap):
    return ap


def _tp(ap):
    return ap.bitcast(TP_DT) if TP_DT is not F32 else ap


def _bn_stats(nc, pool, t_ap, tag, sub=256):
    """Full-row mean/var of [P, C] via bn_stats subgroups. Returns mv [P, 2]."""
    nsub = C // sub
    stats = pool.tile([P, nsub, 6], F32, tag=f"bn_stats_{tag}")
    view = t_ap.rearrange("p (s d) -> p s d", s=nsub)
    for s in range(nsub):
        nc.vector.bn_stats(out=stats[:, s, :], in_=view[:, s, :])
    mv = pool.tile([P, 2], F32, tag=f"bn_mv_{tag}")
    nc.vector.bn_aggr(out=mv, in_=stats)
    return mv


def _rsqrt_newton(nc, pool, v_ap, nb, iters=2):
    """rstd [P, nb] = 1/sqrt(v_ap + EPS), DVE-only (quake seed + Newton)."""
    v = pool.tile([P, nb], F32, tag="rs_v")
    nc.vector.tensor_scalar_add(out=v, in0=v_ap, scalar1=float(EPS))
    sh = pool.tile([P, nb], I32, tag="rs_i")
    nc.vector.tensor_scalar(
        out=sh, in0=v.bitcast(I32), scalar1=1, scalar2=None,
        op0=OP.logical_shift_right,
    )
    seed = pool.tile([P, nb], I32, tag="rs_s")
    nc.vector.tensor_scalar(
        out=seed, in0=sh, scalar1=-1, scalar2=0x5F3759DF,
        op0=OP.mult, op1=OP.add,
    )
    y = seed.bitcast(F32)
    t1 = pool.tile([P, nb], F32, tag="rs_t1")
    t2 = pool.tile([P, nb], F32, tag="rs_t2")
    for _ in range(iters):
        nc.vector.tensor_mul(out=t1, in0=y, in1=y)          # y^2
        nc.vector.tensor_mul(out=t2, in0=t1, in1=v)         # v y^2
        nc.vector.tensor_scalar(                            # 1.5 - 0.5 v y^2
            out=t2, in0=t2, scalar1=-0.5, scalar2=1.5, op0=OP.mult, op1=OP.add,
        )
        nc.vector.tensor_mul(out=t1, in0=y, in1=t2)
        nc.vector.tensor_copy(out=y, in_=t1)
    out = pool.tile([P, nb], F32, tag="rs_out")
    nc.vector.tensor_copy(out=out, in_=y)
    return out


def _normalize(nc, pool, t_ap, mv, rstd, tag):
    """n = (t - mean) * rstd  -> new [P, C] tile (DVE tensor_scalar)."""
    n = pool.tile([P, C], F32, tag=tag)
    nc.vector.tensor_scalar(
        out=n, in0=t_ap, scalar1=mv[:, 0:1], scalar2=rstd,
        op0=OP.subtract, op1=OP.mult,
    )
    return n


def _transpose_768(nc, psum_pool, sb_pool, src_ap, ident, tag):
    """PE-transpose a [P(tokens), C] tile into [P(C-chunk), KC, P(tokens)].

    Result chunk k holds src[:, 128k:128k+128].T; evictions via ScalarE.
    """
    dst = sb_pool.tile([P, KC, P], MM_DT, tag=tag)
    for g in range(2):  # two groups of 3 chunks -> one psum bank each
        tp = psum_pool.tile([P, 3 * P], F32, tag="tp_psum")
        for j in range(3):
            k = 3 * g + j
            nc.tensor.transpose(
                _tp(tp[:, j * P:(j + 1) * P]),
                _tp(src_ap[:, k * P:(k + 1) * P]),
                _tp(ident),
            )
        nc.scalar.copy(out=dst[:, 3 * g:3 * g + 3, :], in_=tp)
    return dst


def build_nc(bshard, flags):
    nc = bacc.Bacc()
    ntiles = bshard // P
    assert bshard % P == 0

    has_c1 = flags["has_c1"]
    has_gb = flags["has_gb"]
    has_bpb = flags["has_bpb"]
    has_c2 = flags["has_c2"]

    xd = nc.dram_tensor("x", [bshard, C], F32, kind="ExternalInput")
    yd = nc.dram_tensor("y", [bshard, C], F32, kind="ExternalInput")
    r1d = nc.dram_tensor("r1", [bshard, C], F32, kind="ExternalInput")
    r2d = nc.dram_tensor("r2", [bshard, C], F32, kind="ExternalInput")
    w1d = nc.dram_tensor("w1", [C, C3], MM_DT, kind="ExternalInput")
    wpbd = nc.dram_tensor("wpb", [HD, HD], MM_DT, kind="ExternalInput")
    w2d = nc.dram_tensor("w2", [C, C], MM_DT, kind="ExternalInput")
    if has_c1:
        c1d = nc.dram_tensor("c1", [C3], F32, kind="ExternalInput")
    if has_gb:
        gd = nc.dram_tensor("lng", [C], F32, kind="ExternalInput")
        bd = nc.dram_tensor("lnb", [C], F32, kind="ExternalInput")
    if has_bpb:
        bpbd = nc.dram_tensor("bpbr", [C], F32, kind="ExternalInput")
    if has_c2:
        c2d = nc.dram_tensor("c2", [C], F32, kind="ExternalInput")

    outs_d = [
        nc.dram_tensor(n, [bshard, C], F32, kind="ExternalOutput")
        for n in ("out_x", "out_y", "out_y2x", "out_x2y")
    ]

    lean = has_c1 or has_gb or has_bpb or has_c2
    with tile.TileContext(nc) as tc:
        with (
            tc.tile_pool(name="wts", bufs=1) as wts,
            tc.tile_pool(name="io", bufs=2) as io,
            tc.tile_pool(name="nt", bufs=1 if lean else 2) as ntp,
            tc.tile_pool(name="qkv", bufs=1 if lean else 2) as qkvp,
            tc.tile_pool(name="blk", bufs=2 if lean else 3) as blk,
            tc.tile_pool(name="tiny", bufs=3) as tiny,
            tc.tile_pool(name="outp", bufs=2) as outp,
            tc.tile_pool(name="psum_tp", bufs=2, space="PSUM") as psum_tp,
            tc.tile_pool(name="psum_pb", bufs=2, space="PSUM") as psum_pb,
            tc.tile_pool(name="psum_mm", bufs=2, space="PSUM") as psum_mm,
        ):
            # ---- persistent weights ----
            w1_sb = []
            for k in range(KC):
                w1_chunk = wts.tile([P, C3], MM_DT, tag=f"w1_{k}")
                w1_sb.append(w1_chunk)
            wpb_sb = wts.tile([P, HD], MM_DT)
            w2_sb = wts.tile([P, KC, C], MM_DT)
            for k in range(KC):
                nc.sync.dma_start(out=w1_sb[k], in_=w1d[k * P:(k + 1) * P, :])
            nc.sync.dma_start(out=wpb_sb[0:HD, :], in_=wpbd[:, :])
            nc.sync.dma_start(out=w2_sb, in_=w2d.rearrange("(k p) c -> p k c", p=P))
            ident = wts.tile([P, P], F32)
            make_identity(nc, ident)

            def bcast_row(src, width, tag):
                t = wts.tile([P, width], F32, tag=tag)
                src_b = bass.AP(
                    tensor=src.tensor, offset=src.offset,
                    ap=[[0, P]] + src.ap,
                )
                nc.gpsimd.dma_start(out=t, in_=src_b)
                return t

            c1_sb = bcast_row(c1d[:], C3, "c1b") if has_c1 else None
            g_sb = bcast_row(gd[:], C, "gb") if has_gb else None
            b_sb = bcast_row(bd[:], C, "bb") if has_gb else None
            bpb_sb = bcast_row(bpbd[:], C, "bpbb") if has_bpb else None
            c2_sb = bcast_row(c2d[:], C, "c2b") if has_c2 else None

            for it in range(ntiles):
                rows = slice(it * P, (it + 1) * P)

                # ---- load + layernorm the four inputs ----
                ins = []
                for nm, d in (("x", xd), ("y", yd), ("r1", r1d), ("r2", r2d)):
                    t = io.tile([P, C], F32, tag=f"in_{nm}")
                    nc.sync.dma_start(out=t, in_=d[rows, :])
                    ins.append(t)

                mvs = [_bn_stats(nc, tiny, t, tag=str(j)) for j, t in enumerate(ins)]
                var4 = tiny.tile([P, 4], F32, tag="var4")
                for j in range(4):
                    nc.vector.tensor_copy(out=var4[:, j:j + 1], in_=mvs[j][:, 1:2])
                rstd4 = _rsqrt_newton(nc, tiny, var4, 4)
                for j, (t, mv) in enumerate(zip(ins, mvs)):
                    nc.vector.tensor_scalar(
                        out=t, in0=t, scalar1=mv[:, 0:1], scalar2=rstd4[:, j:j + 1],
                        op0=OP.subtract, op1=OP.mult,
                    )
                n_in = ins
                n_x, n_y, n_r1, n_r2 = n_in

                # residuals (apply g,b only if present)
                if has_gb:
                    res_in = []
                    for j, n in enumerate(n_in):
                        r = blk.tile([P, C], F32, tag=f"res_{j}")
                        nc.vector.tensor_mul(out=r, in0=n, in1=g_sb)
                        nc.vector.tensor_add(out=r, in0=r, in1=b_sb)
                        res_in.append(r)
                else:
                    res_in = n_in
                res_x, res_y, res_r1, res_r2 = res_in

                # ---- qkv for x and y ----
                nxt = _transpose_768(nc, psum_tp, ntp, n_x, ident, "nxT")
                nyt = _transpose_768(nc, psum_tp, ntp, n_y, ident, "nyT")

                qkv = {}
                for nm, nt in (("x", nxt), ("y", nyt)):
                    q = qkvp.tile([P, C3], F32, tag=f"qkv_{nm}")
                    for nchunk in range(C3 // NCH):
                        ncol = slice(nchunk * NCH, (nchunk + 1) * NCH)
                        ps = psum_mm.tile([P, NCH], F32, tag="mm_psum")
                        for k in range(KC):
                            nc.tensor.matmul(
                                ps, _mm(nt[:, k, :]), _mm(w1_sb[k][:, ncol]),
                                start=(k == 0), stop=(k == KC - 1),
                            )
                        if has_c1:
                            nc.vector.tensor_add(out=q[:, ncol], in0=ps, in1=c1_sb[:, ncol])
                        else:
                            nc.scalar.copy(out=q[:, ncol], in_=ps)
                    qkv[nm] = q

                # ---- four attention blocks ----
                specs = [
                    ("x", "x", res_x, outs_d[0]),
                    ("y", "y", res_y, outs_d[1]),
                    ("y", "x", res_r1, outs_d[2]),
                    ("x", "y", res_r2, outs_d[3]),
                ]
                for bi, (qs, kvs, res, od) in enumerate(specs):
                    qa = qkv[qs][:, 0:C]
                    ka = qkv[kvs][:, C:2 * C]
                    va = qkv[kvs][:, 2 * C:3 * C]

                    e = blk.tile([P, C], F32, tag="e")
                    nc.vector.tensor_mul(out=e, in0=qa, in1=ka)
                    nc.scalar.activation(out=e, in_=e, func=AF.Exp, scale=SCALE)

                    s = tiny.tile([P, H], F32, tag="s_sum")
                    nc.vector.reduce_sum(
                        out=s, in_=e.rearrange("p (h d) -> p h d", h=H), axis=AX.X,
                    )
                    rs = tiny.tile([P, H], F32, tag="s_rec")
                    nc.vector.reciprocal(out=rs, in_=s)

                    nc.vector.tensor_mul(out=e, in0=e, in1=va)  # ev, in place

                    # per-head transpose: evT [96, H, 128] (tokens on free dim)
                    evt = blk.tile([P, H, P], MM_DT, tag="evT")
                    for g in range(2):
                        tp = psum_tp.tile([P, 4 * P], F32, tag="tp_psum_h")
                        for j in range(4):
                            h = 4 * g + j
                            nc.tensor.transpose(
                                _tp(tp[0:HD, j * P:(j + 1) * P]),
                                _tp(e[:, h * HD:(h + 1) * HD]),
                                _tp(ident),
                            )
                        nc.scalar.copy(
                            out=evt[0:HD, 4 * g:4 * g + 4, :], in_=tp[0:HD, :]
                        )

                    u = blk.tile([P, C], F32, tag="u")
                    for g2 in range(2):
                        psg = psum_pb.tile([P, 4 * P], F32, tag="pb_psum")
                        for j in range(4):
                            h = 4 * g2 + j
                            nc.tensor.matmul(
                                psg[:, j * P:j * P + HD],
                                _mm(evt[0:HD, h, :]), _mm(wpb_sb[0:HD, :]),
                                start=True, stop=True,
                            )
                        rs_b = rs[:, 4 * g2:4 * g2 + 4].unsqueeze(2).to_broadcast(
                            (P, 4, HD)
                        )
                        nc.vector.tensor_mul(
                            out=u[:, g2 * NCH:(g2 + 1) * NCH].rearrange(
                                "p (h d) -> p h d", d=HD),
                            in0=psg.rearrange("p (h x) -> p h x", h=4)[:, :, 0:HD],
                            in1=rs_b,
                        )
                    if has_bpb:
                        nc.vector.tensor_add(out=u, in0=u, in1=bpb_sb)
                    nc.vector.tensor_add(out=u, in0=u, in1=res)

                    mv_u = _bn_stats(nc, tiny, u, tag="u")
                    rstd_u = _rsqrt_newton(nc, tiny, mv_u[:, 1:2], 1)
                    nc.vector.tensor_scalar(
                        out=u, in0=u, scalar1=mv_u[:, 0:1], scalar2=rstd_u,
                        op0=OP.subtract, op1=OP.mult,
                    )
                    nut = _transpose_768(nc, psum_tp, blk, u, ident, "nuT")

                    o = outp.tile([P, C], F32, tag="o")
                    for nchunk in range(C // NCH):
                        ncol = slice(nchunk * NCH, (nchunk + 1) * NCH)
                        ps = psum_mm.tile([P, NCH], F32, tag="mm_psum")
                        for k in range(KC):
                            nc.tensor.matmul(
                                ps, _mm(nut[:, k, :]), _mm(w2_sb[:, k, ncol]),
                                start=(k == 0), stop=(k == KC - 1),
                            )
                        if has_c2:
                            nc.vector.tensor_add(out=o[:, ncol], in0=ps, in1=c2_sb[:, ncol])
                        else:
                            nc.scalar.copy(out=o[:, ncol], in_=ps)
                    nc.sync.dma_start(out=od[rows, :], in_=o)

    nc.compile()
    return nc


def _host_prep(x, y, inial_y2x, inial_x2y, Wqkv, Wpb, bpb, Wproj, bproj, ln_g, ln_b):
    g = np.asarray(ln_g, np.float64)
    b = np.asarray(ln_b, np.float64)
    Wqkv64 = np.asarray(Wqkv, np.float64)
    Wproj64 = np.asarray(Wproj, np.float64)

    W1 = (g[:, None] * Wqkv64).astype(np.float32)
    c1 = (b @ Wqkv64).astype(np.float32)
    Wpb_small = np.asarray(Wpb, np.float32)
    bpb_rep = np.tile(np.asarray(bpb, np.float32), H)
    W2 = (g[:, None] * (Wproj64 + np.eye(C))).astype(np.float32)
    c2 = (b @ Wproj64 + np.asarray(bproj, np.float64) + b).astype(np.float32)

    flags = {
        "has_c1": bool(np.any(c1 != 0)),
        "has_gb": bool(np.any(g != 1.0) or np.any(b != 0.0)),
        "has_bpb": bool(np.any(bpb_rep != 0)),
        "has_c2": bool(np.any(c2 != 0)),
    }
    consts = {
        "w1": W1, "wpb": Wpb_small, "w2": W2,
        "c1": c1, "lng": np.asarray(ln_g, np.float32),
        "lnb": np.asarray(ln_b, np.float32),
        "bpbr": bpb_rep, "c2": c2,
    }
    return flags, consts


def _make_in_maps(flags, consts, x, y, r1, r2, n_cores):
    bshard = x.shape[0] // n_cores
    in_maps = []
    for i in range(n_cores):
        sl = slice(i * bshard, (i + 1) * bshard)
        m = {
            "x": np.ascontiguousarray(x[sl], np.float32),
            "y": np.ascontiguousarray(y[sl], np.float32),
            "r1": np.ascontiguousarray(r1[sl], np.float32),
            "r2": np.ascontiguousarray(r2[sl], np.float32),
            "w1": consts["w1"], "wpb": consts["wpb"], "w2": consts["w2"],
        }
        if flags["has_c1"]:
            m["c1"] = consts["c1"]
        if flags["has_gb"]:
            m["lng"], m["lnb"] = consts["lng"], consts["lnb"]
        if flags["has_bpb"]:
            m["bpbr"] = consts["bpbr"]
        if flags["has_c2"]:
            m["c2"] = consts["c2"]
        in_maps.append(m)
    return in_maps, bshard


_KERNEL_CACHE = {}


def kernel(x, y, inial_y2x, inial_x2y, Wqkv, Wpb, bpb, Wproj, bproj, ln_g, ln_b,
           _trace=False):
    x = np.asarray(x, np.float32)
    y = np.asarray(y, np.float32)
    r1 = np.asarray(inial_y2x, np.float32)
    r2 = np.asarray(inial_x2y, np.float32)

    flags, consts = _host_prep(x, y, r1, r2, Wqkv, Wpb, bpb, Wproj, bproj, ln_g, ln_b)
    in_maps, bshard = _make_in_maps(flags, consts, x, y, r1, r2, N_CORES)

    key = (bshard, tuple(sorted(flags.items())))
    if key not in _KERNEL_CACHE:
        _KERNEL_CACHE[key] = build_nc(bshard, flags)
    nc = _KERNEL_CACHE[key]

    res = run_bass_kernel_spmd(nc, in_maps, list(range(N_CORES)), trace=_trace)
    outs = []
    for nm in ("out_x", "out_y", "out_y2x", "out_x2y"):
        outs.append(np.concatenate([res.results[i][nm] for i in range(N_CORES)], axis=0))
    if _trace:
        kernel._last_exec_time_ns = res.exec_time_ns
        kernel._last_results = res
    return tuple(outs)
